# revision 16
# baseline (speedup 1.0000x reference)
"""Trainium2 Bass kernel for one pre-LN transformer block (B=4, T=1024, C=1024,
H=16 heads, FF=4096), distributed over 8 NeuronCores with no collectives.

Sharding: core = (batch b, query-parity j). Each core computes K/V for all 1024
tokens of its batch but attention/FFN only for its 512 queries (tokens t with
t % 2 == j). Interleaved queries make the causal-mask tile structure identical
on every core (SPMD-safe). The host only permutes/transposes inputs and
re-interleaves the outputs.

v5: all PE operands bf16 with 512-wide moving dims (the PE issues at most one
LDWEIGHTS per ~110 ns, so 512-row moving runs are the only full-rate shape;
fp8 DoubleRow at its 256-row cap is LDWEIGHTS-bound and buys nothing). The
optimization targets are the HAM clock gate (a ~1 us PE gap throttles
2.4->1.2 GHz for >10 us) and phase overlap:
 - causal-mask multiply on the idle Pool engine, LN apply's tensor-mul too
 - LayerNorm: G1 = g(x)(-mu*rstd) single rank-1 matmul; beta folded into the
   DVE scalar_tensor_tensor; folded rstd row chain (5 ops)
 - attention tail: pair 7 reciprocal in place on DVE (no DRAM round trip),
   proj split co 0..4 x ci 0..6 before norm(7) so the PE never waits on it
 - proj epilogue interleaved with LN2 stats; LN2 apply interleaved with the
   first 6 ff-blocks of FFN1 (LN G0/G1 run in the "av" PSUM banks)
 - FFN2 keeps W2 resident in SBUF and drains + DMAs each output block as its
   accumulation chain closes (no serial output tail)
"""

import math
import sys
from dataclasses import dataclass

if "/opt/trn_rl_repo" not in sys.path:
    sys.path.insert(0, "/opt/trn_rl_repo")

import numpy as np


@dataclass(frozen=True)
class Cfg:
    B: int = 4
    T: int = 1024
    C: int = 1024
    H: int = 16
    FF: int = 4096

    @property
    def HD(self):
        return self.C // self.H

    @property
    def TQ(self):  # queries per core
        return self.T // 2

    @property
    def NCI(self):  # C / 128 feature tiles
        return self.C // 128

    @property
    def NFF(self):  # FF / 128 hidden tiles
        return self.FF // 128

    @property
    def BW(self):  # token block width for LN1 phases
        return min(512, self.T)

    @property
    def NTB(self):  # token blocks over all T tokens
        return self.T // self.BW

    @property
    def NKB(self):  # key blocks of 128
        return self.T // 128

    def s_kb(self, kb: int) -> int:
        """Start query-column of the computed score region for key block kb."""
        return 128 * (kb % (self.NKB // 2))

    @property
    def pt_offs(self):
        """Column offsets of each key block's packed score region."""
        offs, o = [], 0
        for kb in range(self.NKB):
            offs.append(o)
            o += self.TQ - self.s_kb(kb)
        return offs + [o]


def build_nc(cfg: Cfg, n_cores: int = 8):
    import concourse.tile as tile
    from concourse import bacc, mybir

    f32 = mybir.dt.float32
    f32r = mybir.dt.float32r
    bf16 = mybir.dt.bfloat16
    Act = mybir.ActivationFunctionType
    Alu = mybir.AluOpType

    C, H, HD, FF = cfg.C, cfg.H, cfg.HD, cfg.FF
    NCI, NFF, NKB, NTB = cfg.NCI, cfg.NFF, cfg.NKB, cfg.NTB
    TQ, T = cfg.TQ, cfg.T
    NP = H // 2  # head pairs
    scale = 1.0 / math.sqrt(HD)
    offs = cfg.pt_offs

    nc = bacc.Bacc(
        "TRN2", target_bir_lowering=False, debug=False, num_devices=n_cores
    )

    # ---- DRAM I/O ----
    xpt = nc.dram_tensor("xpt", [C, T], f32r, kind="ExternalInput")
    msk = nc.dram_tensor("msk", [NKB, 128, 128], bf16, kind="ExternalInput")
    dscr_a = nc.dram_tensor("dscr_a", [H * TQ], bf16, kind="Internal")
    dscr_b = nc.dram_tensor("dscr_b", [H * TQ], bf16, kind="Internal")
    wq = nc.dram_tensor("wq", [C, C], bf16, kind="ExternalInput")
    wk = nc.dram_tensor("wk", [C, C], bf16, kind="ExternalInput")
    wv = nc.dram_tensor("wv", [C, C], bf16, kind="ExternalInput")
    wp = nc.dram_tensor("wp", [C, C], bf16, kind="ExternalInput")
    w1 = nc.dram_tensor("w1", [C, FF], bf16, kind="ExternalInput")
    w2 = nc.dram_tensor("w2", [FF, C], bf16, kind="ExternalInput")
    ln1g = nc.dram_tensor("ln1g", [C], f32r, kind="ExternalInput")
    ln1b = nc.dram_tensor("ln1b", [C], f32, kind="ExternalInput")
    ln2g = nc.dram_tensor("ln2g", [C], f32r, kind="ExternalInput")
    ln2b = nc.dram_tensor("ln2b", [C], f32, kind="ExternalInput")
    bpj = nc.dram_tensor("bpj", [C], f32, kind="ExternalInput")
    b1 = nc.dram_tensor("b1", [FF], f32, kind="ExternalInput")
    b2 = nc.dram_tensor("b2", [C], f32, kind="ExternalInput")
    yt = nc.dram_tensor("yt", [C, TQ], f32, kind="ExternalOutput")

    with (
        nc.allow_low_precision(reason="bf16 matmul operands"),
        tile.TileContext(nc) as tc,
    ):
        # ---------------- x DMA first (LN1 starts as soon as possible) ----
        raw, free_raw = tc.tile([128, NCI, T], f32r, name="raw", side="right")
        xpt_r = xpt.rearrange("(ci p) t -> ci p t", p=128)
        for half in range(NTB):
            hsl = slice(half * cfg.BW, (half + 1) * cfg.BW)
            for ci in range(NCI):
                nc.sync.dma_start(out=raw[:, ci, hsl], in_=xpt_r[ci][:, hsl])

        # ---------------- persistent constants / params ----------------
        onesf, free_onesf = tc.tile([128, 512], f32, name="onesf")
        nc.vector.memset(onesf, 1.0)
        ones128, free_ones128 = tc.tile([128, 1], f32r, name="ones128")
        nc.vector.tensor_copy(out=ones128, in_=onesf[:, 0:1])
        ones128b, free_ones128b = tc.tile([128, 1], bf16, name="ones128b")
        nc.vector.tensor_copy(out=ones128b, in_=onesf[:, 0:1])
        # lhsT row of ones at partition 64 for the per-head recip broadcast
        oneshi, free_oneshi = tc.tile([65, HD], bf16, name="oneshi")
        nc.vector.tensor_copy(out=oneshi, in_=onesf[0:65, 0:HD])
        epst, free_epst = tc.tile([1, 1], f32, name="epst")
        nc.vector.memset(epst, 1e-5)

        # LN gammas as [1, C] rows (lhsT of the rank-1 broadcasts); betas as
        # [128, NCI] per-partition columns (folded into the apply's stt).
        g1r, free_g1r = tc.tile([1, C], f32r, name="g1r")
        g2r, free_g2r = tc.tile([1, C], f32r, name="g2r")
        for ptile, v in ((g1r, ln1g), (g2r, ln2g)):
            nc.sync.dma_start(out=ptile, in_=v.rearrange("(o a) -> o a", o=1))
        lb1t, free_lb1t = tc.tile([128, NCI], f32, name="lb1t")
        nc.sync.dma_start(out=lb1t, in_=ln1b.rearrange("(a p) -> p a", p=128))
        lb2t, free_lb2t = tc.tile([128, NCI], f32, name="lb2t")
        nc.sync.dma_start(out=lb2t, in_=ln2b.rearrange("(a p) -> p a", p=128))
        bpjt, free_bpjt = tc.tile([128, NCI], f32, name="bpjt")
        nc.sync.dma_start(out=bpjt, in_=bpj.rearrange("(a p) -> p a", p=128))
        b1t, free_b1t = tc.tile([128, NFF], f32, name="b1t")
        nc.sync.dma_start(out=b1t, in_=b1.rearrange("(a p) -> p a", p=128))
        b2t, free_b2t = tc.tile([128, NCI], f32, name="b2t")
        nc.sync.dma_start(out=b2t, in_=b2.rearrange("(a p) -> p a", p=128))
        mskt, free_mskt = tc.tile([128, NKB, 128], bf16, name="mskt")
        nc.sync.dma_start(out=mskt, in_=msk.rearrange("k p m -> p k m"))

        # PSUM: tag "mm" = 6 rotating banks, tag "av" = 2 banks.
        ps_all = tc.alloc_tile_pool(name="ps_all", bufs=4, space="PSUM")
        wpair = tc.alloc_tile_pool(name="wpair", bufs=3)

        fill_i = [0]

        def emit_fill(n=2, tag="mm"):
            """Dependency-free PE matmuls in ONE psum slot: keep the HAM
            clock gate open through stretches where real PE work is sparse."""
            with nc.named_scope("fill"):
                pw = ps_all.tile([128, 512], f32, tag=tag, bufs=4,
                                 name=f"fw{fill_i[0]}")
                fill_i[0] += 1
                for r in range(n):
                    nc.tensor.matmul(
                        pw, onesf[:, 0:128].bitcast(f32r), onesf.bitcast(f32r),
                        start=(r == 0), stop=(r == n - 1),
                    )

        with nc.named_scope("warmup"):
            for wu in range(11):
                emit_fill(2)

        # x2T = x + attnproj (residual 1), written in the proj phase
        x2t, free_x2t = tc.tile([128, NCI, TQ], bf16, name="x2t")
        # packed normalized heads [128, pair, TQ]
        att2, free_att2 = tc.tile([128, NP, TQ], bf16, name="att2")
        # shared LN scratch (LN1 + LN2) and the attn out-proj weight stream
        ln_sb = tc.alloc_tile_pool(name="ln_sb", bufs=3)
        wp_pool = tc.alloc_tile_pool(name="wp_pool", bufs=5)

        # ---------------- layernorm building blocks ----------------
        def ln_stats_ci(x_ap, psx, psq, ci, ones_x, tag):
            """One ci block's contribution to the mean / sq-mean chains."""
            nc.tensor.matmul(
                psx, ones_x, x_ap, start=(ci == 0), stop=(ci == NCI - 1)
            )
            sq = ln_sb.tile([128, x_ap.shape[-1]], bf16, tag="sq", name=f"sq{tag}_{ci}")
            if ci % 2 == 0:
                nc.scalar.activation(out=sq, in_=x_ap, func=Act.Square)
            else:
                nc.vector.tensor_mul(out=sq, in0=x_ap, in1=x_ap)
            nc.tensor.matmul(
                psq, ones128b, sq, start=(ci == 0), stop=(ci == NCI - 1)
            )

        def ln_rows(psx, psq, blk_w, tag):
            """rstd (c0) and -mu*rstd (c1) rows from the psx/psq sums."""
            ms = ln_sb.tile([1, blk_w], f32r, tag="rs", bufs=4, name=f"ms{tag}")
            nc.vector.tensor_scalar_mul(ms, psq, 1.0 / C)
            mu2 = ln_sb.tile([1, blk_w], f32r, tag="rs", bufs=4, name=f"mu2{tag}")
            nc.scalar.activation(out=mu2, in_=psx, func=Act.Square, scale=1.0 / C)
            nmu = ln_sb.tile([1, blk_w], f32r, tag="rs", bufs=4, name=f"nmu{tag}")
            nc.vector.tensor_scalar_mul(nmu, psx, -1.0 / C)
            var = ln_sb.tile([1, blk_w], f32r, tag="rs", bufs=4, name=f"var{tag}")
            nc.vector.tensor_sub(out=var, in0=ms, in1=mu2)
            sd = ln_sb.tile([1, blk_w], f32r, tag="rs", bufs=4, name=f"sd{tag}")
            nc.scalar.activation(out=sd, in_=var, func=Act.Ln, bias=epst)
            c0 = ln_sb.tile([1, blk_w], f32r, tag=f"c0_{tag}", bufs=1)
            nc.scalar.activation(out=c0, in_=sd, func=Act.Exp, scale=-0.5)
            c1 = ln_sb.tile([1, blk_w], f32r, tag=f"c1_{tag}", bufs=1)
            nc.vector.tensor_mul(out=c1, in0=nmu, in1=c0)
            return c0, c1

        def ln_apply_ci(x_ap, dst_ap, g_row, bcol, ci, c0, c1, tag,
                        ps_tag="mm", mul_pool=False):
            """dst = x*G0 + beta + G1 with G0 = g(x)c0, G1 = g(x)c1."""
            blk_w = x_ap.shape[-1]
            gsl = slice(128 * ci, 128 * (ci + 1))
            G0 = ps_all.tile([128, blk_w], f32, tag=ps_tag, bufs=4,
                             name=f"G0_{tag}_{ci}")
            nc.tensor.matmul(G0, g_row[:, gsl], c0)
            G1 = ps_all.tile([128, blk_w], f32, tag=ps_tag, bufs=4,
                             name=f"G1_{tag}_{ci}")
            nc.tensor.matmul(G1, g_row[:, gsl], c1)
            tmp = ln_sb.tile([128, blk_w], bf16, tag="tmp", name=f"t{tag}_{ci}")
            nc.vector.tensor_mul(out=tmp, in0=x_ap, in1=G0)
            nc.vector.scalar_tensor_tensor(
                out=dst_ap, in0=tmp, scalar=bcol[:, ci : ci + 1], in1=G1,
                op0=Alu.add, op1=Alu.add,
            )

        # ---------------- attention tiles ----------------
        a1, free_a1 = tc.tile([128, NCI, T], bf16, name="a1", side="right")
        # vt: per key block, per head: 64 v-columns + a ones column (fused
        # softmax denominator row in the AV matmul output).
        vt, free_vt = tc.tile([128, NKB, H, HD + 1], bf16, name="vt", side="right")
        for kb in range(NKB):
            nc.vector.tensor_copy(
                out=vt[:, kb, :, HD : HD + 1], in_=onesf[:, 0:H].unsqueeze(2)
            )
        # att holds, per head, O^T rows 0..HD-1 (unnormalized) and the
        # reciprocal softmax denominator in row 64.
        att, free_att = tc.tile([65, H, TQ], bf16, name="att", side="right")

        wq_r = wq.rearrange("(ci p) c -> p ci c", p=128)
        wk_r = wk.rearrange("(ci p) c -> p ci c", p=128)
        wv_r = wv.rearrange("(ci p) c -> p ci c", p=128)
        w1_r = w1.rearrange("(ci p) f -> p ci f", p=128)
        w2_r = w2.rearrange("(fi p) c -> p fi c", p=128)

        qk_pool = tc.alloc_tile_pool(name="qk_pool", bufs=3, side="right")
        pt_pool = tc.alloc_tile_pool(name="pt_pool", bufs=4, side="right")

        qts, kts, pts, avps, wvts = {}, {}, {}, {}, {}
        wqts, wkts = {}, {}

        def emit_qk_dma(hp):
            """Prefetch Q/K weight slices for head pair hp."""
            if hp >= NP:
                return
            csl = slice(128 * hp, 128 * (hp + 1))
            wqt = wpair.tile([128, NCI, 128], bf16, tag="wq", bufs=2, name=f"wq{hp}")
            nc.sync.dma_start(out=wqt, in_=wq_r[:, :, csl])
            wkt = wpair.tile([128, NCI, 128], bf16, tag="wk", bufs=2, name=f"wk{hp}")
            nc.sync.dma_start(out=wkt, in_=wk_r[:, :, csl])
            wqts[hp], wkts[hp] = wqt, wkt

        def emit_q(hp):
            """Q projection for head pair hp (feature rows 128*hp..)."""
            if hp >= NP:
                return
            with nc.named_scope("qkv"):
                qt = qk_pool.tile([128, TQ], bf16, tag="qt", name=f"qt{hp}")
                pq = ps_all.tile([128, TQ], f32, tag="mm", name=f"pq{hp}")
                for ci in range(NCI):
                    nc.tensor.matmul(
                        pq, wqts[hp][:, ci, :], a1[:, ci, 0:TQ],
                        start=(ci == 0), stop=(ci == NCI - 1),
                    )
                nc.scalar.copy(out=qt, in_=pq)
                qts[hp] = qt

        def emit_k(hp, tb):
            """K projection for head pair hp, token half tb."""
            if hp >= NP:
                return
            with nc.named_scope("qkv"):
                if tb == 0:
                    kts[hp] = qk_pool.tile([128, T], bf16, tag="kt", name=f"kt{hp}")
                sl = slice(512 * tb, 512 * (tb + 1))
                pk = ps_all.tile([128, 512], f32, tag="mm", name=f"pk{hp}_{tb}")
                for ci in range(NCI):
                    nc.tensor.matmul(
                        pk, wkts[hp][:, ci, :], a1[:, ci, sl],
                        start=(ci == 0), stop=(ci == NCI - 1),
                    )
                nc.scalar.copy(out=kts[hp][:, sl], in_=pk)

        def emit_vdma(g):
            """Prefetch the V weight slice for heads 4g..4g+3."""
            if g >= H // 4:
                return
            csl = slice(256 * g, 256 * (g + 1))
            wvt = wpair.tile([128, NCI, 256], bf16, tag="wv", bufs=2, name=f"wv{g}")
            nc.sync.dma_start(out=wvt, in_=wv_r[:, :, csl])
            wvts[g] = wvt

        def emit_vchunk_kb(g, kb):
            """V projection for heads 4g..4g+3 (pairs 2g, 2g+1), one key
            block. Activations stationary, 256 weight columns moving."""
            if g >= H // 4:
                return
            with nc.named_scope("qkv"):
                kbsl = slice(128 * kb, 128 * (kb + 1))
                pv = ps_all.tile([128, 256], f32, tag="mm", name=f"pv{g}_{kb}")
                for ci in range(NCI):
                    nc.tensor.matmul(
                        pv, a1[:, ci, kbsl], wvts[g][:, ci, :],
                        start=(ci == 0), stop=(ci == NCI - 1),
                    )
                nc.vector.tensor_copy(
                    out=vt[:, kb, 4 * g : 4 * g + 4, 0:HD],
                    in_=pv.rearrange("p (h d) -> p h d", h=4),
                )

        def emit_scores_kb(hp, kb):
            """Scores + exp + causal mask for both heads of pair hp, one
            key block. Mask multiply runs on the Pool engine."""
            if hp >= NP:
                return
            with nc.named_scope("attn"):
                if kb == 0:
                    p0 = pt_pool.tile([128, offs[-1]], bf16, tag="pt", name=f"pt{2 * hp}")
                    p1 = pt_pool.tile([128, offs[-1]], bf16, tag="pt", name=f"pt{2 * hp + 1}")
                    pts[hp] = (p0, p1)
                qt, kt = qts[hp], kts[hp]
                s = cfg.s_kb(kb)
                n = TQ - s
                kbsl = slice(128 * kb, 128 * (kb + 1))
                pss = []
                for idx in range(2):
                    po = idx * HD
                    ps_s = ps_all.tile([128, 512], f32, tag="mm", name=f"sc{hp}_{kb}_{idx}")
                    nc.tensor.matmul(
                        ps_s[:, 0:n],
                        kt[po : po + HD, kbsl],
                        qt[po : po + HD, s:TQ],
                    )
                    pss.append(ps_s)
                for idx in range(2):
                    dst = pts[hp][idx]
                    nc.scalar.activation(
                        out=dst[:, offs[kb] : offs[kb] + n],
                        in_=pss[idx][:, 0:n],
                        func=Act.Exp, scale=scale,
                    )
                    nc.gpsimd.tensor_mul(
                        out=dst[:, offs[kb] : offs[kb] + 128],
                        in0=dst[:, offs[kb] : offs[kb] + 128],
                        in1=mskt[:, kb, :],
                    )

        def emit_av_kb(hp, kb):
            """One key block of the AV accumulation for both heads of pair
            hp (inputs were produced one pair-period earlier)."""
            if hp < 0:
                return
            with nc.named_scope("attn"):
                s = cfg.s_kb(kb)
                for idx in range(2):
                    h = 2 * hp + idx
                    if kb == 0:
                        avps[h] = ps_all.tile(
                            [65, TQ], f32, tag="av", bufs=4, name=f"av{h}"
                        )
                    nc.tensor.matmul(
                        avps[h][:, s:TQ],
                        vt[:, kb, h, :],
                        pts[hp][idx][:, offs[kb] : offs[kb + 1]],
                        start=(kb == 0), stop=(kb == NKB - 1),
                        skip_group_check=True,
                    )

        def emit_av_finish(hp):
            """Copy unnormalized O^T and denominator rows out of PSUM."""
            with nc.named_scope("attn"):
                for idx in range(2):
                    h = 2 * hp + idx
                    nc.scalar.copy(out=att[0:64, h, :], in_=avps[h][0:64, :])
                    nc.vector.tensor_copy(
                        out=att[64:65, h, :], in_=avps[h][64:65, :]
                    )

        def emit_recip(hs, nh):
            """Batch-reciprocal the denominator rows of heads hs..hs+nh-1
            via a DRAM round-trip spreading them over 128 partitions."""
            assert (nh * TQ) % 128 == 0
            with nc.named_scope("attn"):
                hsl = slice(hs, hs + nh)
                fl = nh * TQ // 128
                nc.sync.dma_start(
                    out=dscr_a.rearrange("(o h t) -> o h t", o=1, h=H)[:, hsl, :],
                    in_=att[64:65, hsl, :],
                )
                dwide = pt_pool.tile([128, fl], bf16, tag="dw", bufs=2, name=f"dw{hs}")
                nc.sync.dma_start(
                    out=dwide,
                    in_=dscr_a[hs * TQ : (hs + nh) * TQ].rearrange(
                        "(p f) -> p f", p=128
                    ),
                )
                nc.vector.reciprocal(out=dwide, in_=dwide)
                nc.sync.dma_start(
                    out=dscr_b[hs * TQ : (hs + nh) * TQ].rearrange(
                        "(p f) -> p f", p=128
                    ),
                    in_=dwide,
                )
                nc.sync.dma_start(
                    out=att[64:65, hsl, :],
                    in_=dscr_b.rearrange("(o h t) -> o h t", o=1, h=H)[:, hsl, :],
                )

        def emit_norm(hp):
            """Normalize pair hp's heads by the reciprocal denominators and
            pack them into att2[:, hp, :] (odd head via partition-shifted
            DVE write to partitions 64..127)."""
            with nc.named_scope("attn"):
                for idx in range(2):
                    h = 2 * hp + idx
                    bc = ps_all.tile([64, TQ], f32, tag="mm", name=f"bc{h}")
                    nc.tensor.matmul(bc, oneshi[64:65, :], att[64:65, h, :])
                    psl = slice(64 * idx, 64 * idx + 64)
                    nc.vector.tensor_mul(
                        out=att2[psl, hp, :], in0=att[0:64, h, :], in1=bc
                    )

        # pipeline: scores of pair hp+1, AV of pair hp, V of pair hp+1,
        # and Q/K of pair hp+2 are interleaved at key-block granularity.
        def ln1_post_block(tb):
            if tb == 0:
                emit_qk_dma(0)
                emit_qk_dma(1)
                emit_vdma(0)
                emit_q(0)
                emit_k(0, 0)
                for kb in range(NKB // 2):
                    emit_scores_kb(0, kb)
                    emit_vchunk_kb(0, kb)
            else:
                emit_k(0, 1)
                for kb in range(NKB // 2, NKB):
                    emit_scores_kb(0, kb)
                    emit_vchunk_kb(0, kb)
                emit_q(1)
                emit_k(1, 0)
                emit_k(1, 1)

        # ---------------- LN1 over all T tokens ----------------
        with nc.named_scope("ln1"):
            stats1 = []
            for tb in range(NTB):
                sl = slice(tb * cfg.BW, (tb + 1) * cfg.BW)
                stag = "mm" if tb == 0 else "av"
                psx = ps_all.tile([1, cfg.BW], f32, tag=stag, bufs=4, name=f"psx{tb}")
                psq = ps_all.tile([1, cfg.BW], f32, tag=stag, bufs=4, name=f"psq{tb}")
                for ci in range(NCI):
                    ln_stats_ci(
                        raw[:, ci, sl], psx, psq, ci, ones128, f"1{tb}"
                    )
                stats1.append((psx, psq))
                emit_fill(2)
            emit_fill(14, tag="av")
            rows1 = []
            for tb in range(NTB):
                psx, psq = stats1[tb]
                rows1.append(ln_rows(psx, psq, cfg.BW, f"1{tb}"))
            for tb in range(NTB):
                sl = slice(tb * cfg.BW, (tb + 1) * cfg.BW)
                c0, c1 = rows1[tb]
                for ci in range(NCI):
                    ln_apply_ci(
                        raw[:, ci, sl], a1[:, ci, sl], g1r, lb1t,
                        ci, c0, c1, f"1{tb}",
                    )
                    emit_fill(2)
                ln1_post_block(tb)

        # ---------------- attention pair loop ----------------
        wpts = {}
        for hp in range(NP):
            g = (hp + 1) // 2
            vchunk = hp % 2 == 1 and g < H // 4
            if vchunk:
                emit_vdma(g)
            for kb in range(NKB):
                emit_scores_kb(hp + 1, kb)
                emit_av_kb(hp, kb)
                if vchunk:
                    emit_vchunk_kb(g, kb)
                if hp == 7:
                    emit_fill(2)
                if kb == 1:
                    emit_qk_dma(hp + 2)
                elif kb == 3:
                    emit_q(hp + 2)
                    if 1 <= hp < 7:
                        emit_norm(hp - 1)
                elif kb == 6 and hp == 7:
                    emit_norm(6)
                elif kb == 5:
                    emit_k(hp + 2, 0)
                elif kb == 7:
                    emit_k(hp + 2, 1)
                # prefetch attn out-proj weights during the late pairs
                if hp >= 5 and kb % 2 == 1:
                    ci = (hp - 5) * 4 + kb // 2
                    if ci < NCI:
                        wt = wp_pool.tile([128, C], bf16, tag="w", name=f"wpt{ci}")
                        nc.sync.dma_start(
                            out=wt, in_=wp[128 * ci : 128 * (ci + 1)]
                        )
                        wpts[ci] = wt
            if hp < 7:
                emit_av_finish(hp)
                emit_recip(2 * hp, 2)

        # ---------------- attention tail + out-proj + residual 1 ----------
        # Pair 7: O rows drain via ACT; reciprocal straight off the PSUM
        # denominator rows on DVE (no DRAM round-trip).
        with nc.named_scope("attn"):
            for idx in range(2):
                h = 14 + idx
                nc.scalar.copy(out=att[0:64, h, :], in_=avps[h][0:64, :])
                nc.vector.reciprocal(
                    out=att[64:65, h, :], in_=avps[h][64:65, :]
                )

        # proj part A: co 0..4 x ci 0..6 keeps the PE busy while pair 7's
        # reciprocal + norm resolve (pp[0..4] on "mm"; one mm slot stays
        # free for norm(7)'s bc; pp[5..7] allocated after the "av" banks
        # drain).
        pp = [None] * NCI
        with nc.named_scope("proj"):
            for i in range(4):
                pp[i] = ps_all.tile([128, TQ], f32, tag="mm", bufs=4, name=f"pp{i}")
            for ci in range(7):
                for co in range(4):
                    nc.tensor.matmul(
                        pp[co],
                        wpts[ci][:, 128 * co : 128 * (co + 1)],
                        att2[:, ci, :],
                        start=(ci == 0), stop=False,
                    )
        # norm(7): bc matmuls run in the (already drained) "av" banks so the
        # six open pp accumulators in "mm" are never wrapped onto.
        with nc.named_scope("attn"):
            for idx in range(2):
                h = 14 + idx
                bc = ps_all.tile([64, TQ], f32, tag="av", bufs=4, name=f"bc{h}")
                nc.tensor.matmul(bc, oneshi[64:65, :], att[64:65, h, :])
                psl = slice(64 * idx, 64 * idx + 64)
                nc.vector.tensor_mul(
                    out=att2[psl, 7, :], in0=att[0:64, h, :], in1=bc
                )
        with nc.named_scope("proj"):
            for i in range(4, 8):
                pp[i] = ps_all.tile([128, TQ], f32, tag="av", bufs=4, name=f"pp{i}")
            # finish: ci 0..6 for co 4..7, then ci 7 for all co
            for ci in range(7):
                for co in range(4, 8):
                    nc.tensor.matmul(
                        pp[co],
                        wpts[ci][:, 128 * co : 128 * (co + 1)],
                        att2[:, ci, :],
                        start=(ci == 0), stop=False,
                    )
            for co in range(NCI):
                nc.tensor.matmul(
                    pp[co],
                    wpts[7][:, 128 * co : 128 * (co + 1)],
                    att2[:, 7, :],
                    start=False, stop=True,
                )
        wp_pool.release()
        w1s = tc.alloc_tile_pool(name="w1s", bufs=4)

        # w1 prefetch for the LN2-interleaved first FFN1 group (6 ff-blocks)
        w1ts = {}
        for ci in range(3):
            w1t = w1s.tile([128, 512], bf16, tag="w1", bufs=6, name=f"w1g0_{ci}")
            nc.sync.dma_start(out=w1t, in_=w1_r[:, ci, 0:512])
            w1ts[(0, ci)] = w1t

        # proj epilogue interleaved with LN2 stats (per ci as x2t lands)
        psx2 = ps_all.tile([1, TQ], f32, tag="mm", name="psx2")
        psq2 = ps_all.tile([1, TQ], f32, tag="mm", name="psq2")
        def emit_x2t(co):
            nc.vector.scalar_tensor_tensor(
                out=x2t[:, co, :],
                in0=pp[co],
                scalar=bpjt[:, co : co + 1],
                in1=raw[:, co, 0:TQ],
                op0=Alu.add,
                op1=Alu.add,
            )

        with nc.named_scope("proj"):
            emit_x2t(0)
            emit_x2t(1)
            for co in range(NCI):
                if co + 2 < NCI:
                    emit_x2t(co + 2)
                with nc.named_scope("ln2"):
                    ln_stats_ci(
                        x2t[:, co, :], psx2, psq2, co, ones128b, "2"
                    )
                if co >= 3:
                    w1t = w1s.tile(
                        [128, 512], bf16, tag="w1", bufs=6, name=f"w1g0_{co}"
                    )
                    nc.sync.dma_start(out=w1t, in_=w1_r[:, co, 0:512])
                    w1ts[(0, co)] = w1t
                if co == 4:
                    emit_fill(22, tag="av")
        pt_pool.release()
        qk_pool.release()
        free_att()
        free_vt()
        free_a1()
        free_raw()

        # ---------------- LN2 rows + apply interleaved with FFN1 group 0 --
        a2, free_a2 = tc.tile([128, NCI, TQ], bf16, name="a2", side="right")
        hsb, free_hsb = tc.tile([128, NFF, TQ], bf16, name="hsb", side="right")
        w2s = tc.alloc_tile_pool(name="w2s", bufs=4)
        with nc.named_scope("ln2"):
            c02, c12 = ln_rows(psx2, psq2, TQ, "2")
            emit_fill(4)
        # FFN1 group 0: ff-blocks 0..5 accumulate per-ci right after each
        # LN2 apply lands that ci; LN G0/G1 run in the "av" banks.
        pf0 = [
            ps_all.tile([128, TQ], f32, tag="mm", name=f"pf0_{i}")
            for i in range(6)
        ]
        with nc.named_scope("ln2"):
            for ci in range(NCI):
                ln_apply_ci(
                    x2t[:, ci, :], a2[:, ci, :], g2r, lb2t,
                    ci, c02, c12, "2", ps_tag="av",
                )
                with nc.named_scope("ffn1"):
                    for co in range(4):
                        nc.tensor.matmul(
                            pf0[co],
                            w1ts[(0, ci)][:, 128 * co : 128 * (co + 1)],
                            a2[:, ci, :],
                            start=(ci == 0), stop=(ci == NCI - 1),
                        )
        x3t, free_x3t = tc.tile([128, NCI, TQ], bf16, name="x3t")
        w2ts = {}
        with nc.named_scope("ffn1"):
            for co in range(4):
                nc.scalar.activation(
                    out=hsb[:, co, :], in_=pf0[co], func=Act.Gelu,
                    bias=b1t[:, co : co + 1],
                )
            # remaining 28 ff-blocks in groups; weights streamed per (g, ci)
            groups = [(4, 8), (12, 8), (20, 8), (28, 4)]
            for gi, (f0, nco) in enumerate(groups):
                pf = [
                    ps_all.tile(
                        [128, TQ], f32,
                        tag=("mm" if i < 4 else "av"), bufs=4,
                        name=f"pf{gi + 1}_{i}",
                    )
                    for i in range(nco)
                ]
                for ci in range(NCI):
                    wt = w1s.tile(
                        [128, 128 * nco], bf16, tag="w1", bufs=6,
                        name=f"w1g{gi + 1}_{ci}",
                    )
                    nc.sync.dma_start(
                        out=wt,
                        in_=w1_r[:, ci, 128 * f0 : 128 * (f0 + nco)],
                    )
                    for co in range(nco):
                        nc.tensor.matmul(
                            pf[co],
                            wt[:, 128 * co : 128 * (co + 1)],
                            a2[:, ci, :],
                            start=(ci == 0), stop=(ci == NCI - 1),
                        )
                for co in range(nco):
                    hco = f0 + co
                    nc.scalar.activation(
                        out=hsb[:, hco, :], in_=pf[co], func=Act.Gelu,
                        bias=b1t[:, hco : hco + 1],
                    )
                # W2 prefetch + x3t prep spread across the groups
                w2t = w2s.tile([128, 8, C], bf16, tag="w2", bufs=4, name=f"w2t{gi}")
                nc.sync.dma_start(out=w2t, in_=w2_r[:, 8 * gi : 8 * (gi + 1), :])
                w2ts[gi] = w2t
                for k in range(2):
                    ci = 2 * gi + k
                    nc.vector.tensor_scalar_add(
                        x3t[:, ci, :], x2t[:, ci, :], b2t[:, ci : ci + 1]
                    )

        # ---------------- FFN2: resident W2, output-pipelined ----------
        yt_pool = tc.alloc_tile_pool(name="yt_pool", bufs=3, side="right")
        yt_r = yt.rearrange("(ci p) t -> ci p t", p=128)
        with nc.named_scope("ffn2"):
            for co in range(NCI):
                py = ps_all.tile(
                    [128, TQ], f32,
                    tag=("mm" if co % 2 == 0 else "av"), bufs=4,
                    name=f"py{co}",
                )
                csl = slice(128 * co, 128 * (co + 1))
                for fi in range(NFF):
                    nc.tensor.matmul(
                        py,
                        w2ts[fi // 8][:, fi % 8, csl],
                        hsb[:, fi, :],
                        start=(fi == 0), stop=(fi == NFF - 1),
                    )
                yts = yt_pool.tile([128, TQ], f32, tag="y", name=f"yts{co}")
                nc.vector.tensor_add(out=yts, in0=py, in1=x3t[:, co, :])
                nc.sync.dma_start(out=yt_r[co], in_=yts)

        yt_pool.release()
        free_x3t()
        w2s.release()
        free_hsb()
        free_a2()
        w1s.release()
        ln_sb.release()
        free_att2()
        free_x2t()
        wpair.release()
        ps_all.release()
        free_mskt()
        free_b2t()
        free_b1t()
        free_bpjt()
        free_lb2t()
        free_lb1t()
        free_g2r()
        free_g1r()
        free_epst()
        free_oneshi()
        free_ones128b()
        free_ones128()
        free_onesf()

    nc.compile()
    return nc


def prep_core_inputs(cfg: Cfg, inputs: dict, b: int, j: int) -> dict:
    """Host-side slicing/permutation for core (batch b, parity j)."""
    T, TQ, NKB = cfg.T, cfg.TQ, cfg.NKB
    x = np.asarray(inputs["x"])
    perm = np.concatenate([np.arange(j, T, 2), np.arange(1 - j, T, 2)])
    xp = x[b][perm]  # [T, C]
    xpt = np.ascontiguousarray(xp.T, dtype=np.float32)

    import ml_dtypes

    qtok = perm[:TQ]
    ktok = perm
    mask = np.ones((NKB, 128, 128), dtype=np.float32)
    for kb in range(NKB):
        s = cfg.s_kb(kb)
        kt = ktok[128 * kb : 128 * (kb + 1)]  # [128]
        qt = qtok[s : s + 128]  # [128]
        allowed = qt[None, :] >= kt[:, None]  # [128, 128]
        mask[kb] = np.where(allowed, 1.0, 0.0)
    return {"xpt": xpt, "msk": mask.astype(ml_dtypes.bfloat16)}


def prep_shared_inputs(cfg: Cfg, inputs: dict) -> dict:
    import ml_dtypes

    C = cfg.C
    f32 = np.float32
    bf16 = ml_dtypes.bfloat16

    def wq2d(w):  # [H, C, HD] -> [C, H*HD]
        w = np.asarray(w)
        return np.ascontiguousarray(
            w.transpose(1, 0, 2).reshape(C, C)
        ).astype(bf16)

    return {
        "wq": wq2d(inputs["Wq"]),
        "wk": wq2d(inputs["Wk"]),
        "wv": wq2d(inputs["Wv"]),
        "wp": np.ascontiguousarray(inputs["Wproj"]).astype(bf16),
        "w1": np.ascontiguousarray(inputs["W1"]).astype(bf16),
        "w2": np.ascontiguousarray(inputs["W2"]).astype(bf16),
        "ln1g": np.ascontiguousarray(inputs["ln1_g"], dtype=f32),
        "ln1b": np.ascontiguousarray(inputs["ln1_b"], dtype=f32),
        "ln2g": np.ascontiguousarray(inputs["ln2_g"], dtype=f32),
        "ln2b": np.ascontiguousarray(inputs["ln2_b"], dtype=f32),
        "bpj": np.ascontiguousarray(inputs["bproj"], dtype=f32),
        "b1": np.ascontiguousarray(inputs["b1"], dtype=f32),
        "b2": np.ascontiguousarray(inputs["b2"], dtype=f32),
    }


def run(cfg: Cfg, inputs: dict, n_cores: int = 8, trace: bool = False):
    from concourse.bass_utils import run_bass_kernel_spmd

    nc = build_nc(cfg, n_cores=n_cores)
    shared = prep_shared_inputs(cfg, inputs)
    in_maps = []
    cores = []
    for core in range(n_cores):
        b, j = divmod(core, 2)
        b = b % cfg.B
        in_maps.append({**prep_core_inputs(cfg, inputs, b, j), **shared})
        cores.append((b, j))
    res = run_bass_kernel_spmd(
        nc, in_maps, core_ids=list(range(n_cores)), trace=trace
    )
    out = np.zeros((cfg.B, cfg.T, cfg.C), dtype=np.float32)
    for core, (b, j) in enumerate(cores):
        ytv = res.results[core]["yt"]  # [C, TQ]
        perm = np.concatenate(
            [np.arange(j, cfg.T, 2), np.arange(1 - j, cfg.T, 2)]
        )
        out[b, perm[: cfg.TQ], :] = ytv.T
    return out, res


def kernel(**inputs) -> np.ndarray:
    out, _ = run(Cfg(), inputs, n_cores=8, trace=False)
    return out


if __name__ == "__main__":
    rng = np.random.default_rng(0)
    cfg = Cfg()
    ins = {
        "x": rng.standard_normal((cfg.B, cfg.T, cfg.C)).astype(np.float32),
        "ln1_g": np.ones(cfg.C, np.float32),
        "ln1_b": np.zeros(cfg.C, np.float32),
        "ln2_g": np.ones(cfg.C, np.float32),
        "ln2_b": np.zeros(cfg.C, np.float32),
        "Wq": rng.standard_normal((cfg.H, cfg.C, cfg.HD)).astype(np.float32)
        * 0.02,
        "Wk": rng.standard_normal((cfg.H, cfg.C, cfg.HD)).astype(np.float32)
        * 0.02,
        "Wv": rng.standard_normal((cfg.H, cfg.C, cfg.HD)).astype(np.float32)
        * 0.02,
        "Wproj": rng.standard_normal((cfg.C, cfg.C)).astype(np.float32) * 0.02,
        "bproj": np.zeros(cfg.C, np.float32),
        "W1": rng.standard_normal((cfg.C, cfg.FF)).astype(np.float32) * 0.02,
        "b1": np.zeros(cfg.FF, np.float32),
        "W2": rng.standard_normal((cfg.FF, cfg.C)).astype(np.float32) * 0.02,
        "b2": np.zeros(cfg.C, np.float32),
    }
    y = kernel(**ins)
    print("ran, out", y.shape, y.dtype, float(np.abs(y).max()))


# revision 20
# speedup vs baseline: 1.0132x; 1.0132x over previous
"""Trainium2 Bass kernel for one pre-LN transformer block (B=4, T=1024, C=1024,
H=16 heads, FF=4096), distributed over 8 NeuronCores with no collectives.

Sharding: core = (batch b, query-parity j). Each core computes K/V for all 1024
tokens of its batch but attention/FFN only for its 512 queries (tokens t with
t % 2 == j). Interleaved queries make the causal-mask tile structure identical
on every core (SPMD-safe). The host only permutes/transposes inputs and
re-interleaves the outputs.

v5: all PE operands bf16 with 512-wide moving dims (the PE issues at most one
LDWEIGHTS per ~110 ns, so 512-row moving runs are the only full-rate shape;
fp8 DoubleRow at its 256-row cap is LDWEIGHTS-bound and buys nothing). The
optimization targets are the HAM clock gate (a ~1 us PE gap throttles
2.4->1.2 GHz for >10 us) and phase overlap:
 - causal-mask multiply on the idle Pool engine, LN apply's tensor-mul too
 - LayerNorm: G1 = g(x)(-mu*rstd) single rank-1 matmul; beta folded into the
   DVE scalar_tensor_tensor; folded rstd row chain (5 ops)
 - attention tail: pair 7 reciprocal in place on DVE (no DRAM round trip),
   proj split co 0..4 x ci 0..6 before norm(7) so the PE never waits on it
 - proj epilogue interleaved with LN2 stats; LN2 apply interleaved with the
   first 6 ff-blocks of FFN1 (LN G0/G1 run in the "av" PSUM banks)
 - FFN2 keeps W2 resident in SBUF and drains + DMAs each output block as its
   accumulation chain closes (no serial output tail)
"""

import math
import sys
from dataclasses import dataclass

if "/opt/trn_rl_repo" not in sys.path:
    sys.path.insert(0, "/opt/trn_rl_repo")

import numpy as np


@dataclass(frozen=True)
class Cfg:
    B: int = 4
    T: int = 1024
    C: int = 1024
    H: int = 16
    FF: int = 4096

    @property
    def HD(self):
        return self.C // self.H

    @property
    def TQ(self):  # queries per core
        return self.T // 2

    @property
    def NCI(self):  # C / 128 feature tiles
        return self.C // 128

    @property
    def NFF(self):  # FF / 128 hidden tiles
        return self.FF // 128

    @property
    def BW(self):  # token block width for LN1 phases
        return min(512, self.T)

    @property
    def NTB(self):  # token blocks over all T tokens
        return self.T // self.BW

    @property
    def NKB(self):  # key blocks of 128
        return self.T // 128

    def s_kb(self, kb: int) -> int:
        """Start query-column of the computed score region for key block kb."""
        return 128 * (kb % (self.NKB // 2))

    @property
    def pt_offs(self):
        """Column offsets of each key block's packed score region."""
        offs, o = [], 0
        for kb in range(self.NKB):
            offs.append(o)
            o += self.TQ - self.s_kb(kb)
        return offs + [o]


def build_nc(cfg: Cfg, n_cores: int = 8):
    import concourse.tile as tile
    from concourse import bacc, mybir

    f32 = mybir.dt.float32
    f32r = mybir.dt.float32r
    bf16 = mybir.dt.bfloat16
    Act = mybir.ActivationFunctionType
    Alu = mybir.AluOpType

    C, H, HD, FF = cfg.C, cfg.H, cfg.HD, cfg.FF
    NCI, NFF, NKB, NTB = cfg.NCI, cfg.NFF, cfg.NKB, cfg.NTB
    TQ, T = cfg.TQ, cfg.T
    NP = H // 2  # head pairs
    scale = 1.0 / math.sqrt(HD)
    offs = cfg.pt_offs

    nc = bacc.Bacc(
        "TRN2", target_bir_lowering=False, debug=False, num_devices=n_cores
    )

    # ---- DRAM I/O ----
    xpt = nc.dram_tensor("xpt", [C, T], f32r, kind="ExternalInput")
    msk = nc.dram_tensor("msk", [NKB, 128, 128], bf16, kind="ExternalInput")
    dscr_a = nc.dram_tensor("dscr_a", [H * TQ], bf16, kind="Internal")
    dscr_b = nc.dram_tensor("dscr_b", [H * TQ], bf16, kind="Internal")
    wq = nc.dram_tensor("wq", [C, C], bf16, kind="ExternalInput")
    wk = nc.dram_tensor("wk", [C, C], bf16, kind="ExternalInput")
    wv = nc.dram_tensor("wv", [C, C], bf16, kind="ExternalInput")
    wp = nc.dram_tensor("wp", [C, C], bf16, kind="ExternalInput")
    w1 = nc.dram_tensor("w1", [C, FF], bf16, kind="ExternalInput")
    w2 = nc.dram_tensor("w2", [FF, C], bf16, kind="ExternalInput")
    ln1g = nc.dram_tensor("ln1g", [C], f32r, kind="ExternalInput")
    ln1b = nc.dram_tensor("ln1b", [C], f32, kind="ExternalInput")
    ln2g = nc.dram_tensor("ln2g", [C], f32r, kind="ExternalInput")
    ln2b = nc.dram_tensor("ln2b", [C], f32, kind="ExternalInput")
    bpj = nc.dram_tensor("bpj", [C], f32, kind="ExternalInput")
    b1 = nc.dram_tensor("b1", [FF], f32, kind="ExternalInput")
    b2 = nc.dram_tensor("b2", [C], f32, kind="ExternalInput")
    yt = nc.dram_tensor("yt", [C, TQ], f32, kind="ExternalOutput")

    with (
        nc.allow_low_precision(reason="bf16 matmul operands"),
        tile.TileContext(nc) as tc,
    ):
        # ---------------- x DMA first (LN1 starts as soon as possible) ----
        raw, free_raw = tc.tile([128, NCI, T], f32r, name="raw", side="right")
        xpt_r = xpt.rearrange("(ci p) t -> ci p t", p=128)
        for half in range(NTB):
            hsl = slice(half * cfg.BW, (half + 1) * cfg.BW)
            for ci in range(NCI):
                nc.sync.dma_start(out=raw[:, ci, hsl], in_=xpt_r[ci][:, hsl])

        # ---------------- persistent constants / params ----------------
        onesf, free_onesf = tc.tile([128, 512], f32, name="onesf")
        nc.vector.memset(onesf, 1.0)
        ones128, free_ones128 = tc.tile([128, 1], f32r, name="ones128")
        nc.vector.tensor_copy(out=ones128, in_=onesf[:, 0:1])
        ones128b, free_ones128b = tc.tile([128, 1], bf16, name="ones128b")
        nc.vector.tensor_copy(out=ones128b, in_=onesf[:, 0:1])
        # lhsT row of ones at partition 64 for the per-head recip broadcast
        oneshi, free_oneshi = tc.tile([65, HD], bf16, name="oneshi")
        nc.vector.tensor_copy(out=oneshi, in_=onesf[0:65, 0:HD])
        epst, free_epst = tc.tile([1, 1], f32, name="epst")
        nc.vector.memset(epst, 1e-5)

        # LN gammas as [1, C] rows (lhsT of the rank-1 broadcasts); betas as
        # [128, NCI] per-partition columns (folded into the apply's stt).
        g1r, free_g1r = tc.tile([1, C], f32r, name="g1r")
        g2r, free_g2r = tc.tile([1, C], f32r, name="g2r")
        for ptile, v in ((g1r, ln1g), (g2r, ln2g)):
            nc.sync.dma_start(out=ptile, in_=v.rearrange("(o a) -> o a", o=1))
        lb1t, free_lb1t = tc.tile([128, NCI], f32, name="lb1t")
        nc.sync.dma_start(out=lb1t, in_=ln1b.rearrange("(a p) -> p a", p=128))
        lb2t, free_lb2t = tc.tile([128, NCI], f32, name="lb2t")
        nc.sync.dma_start(out=lb2t, in_=ln2b.rearrange("(a p) -> p a", p=128))
        bpjt, free_bpjt = tc.tile([128, NCI], f32, name="bpjt")
        nc.sync.dma_start(out=bpjt, in_=bpj.rearrange("(a p) -> p a", p=128))
        b1t, free_b1t = tc.tile([128, NFF], f32, name="b1t")
        nc.sync.dma_start(out=b1t, in_=b1.rearrange("(a p) -> p a", p=128))
        b2t, free_b2t = tc.tile([128, NCI], f32, name="b2t")
        nc.sync.dma_start(out=b2t, in_=b2.rearrange("(a p) -> p a", p=128))
        mskt, free_mskt = tc.tile([128, NKB, 128], bf16, name="mskt")
        nc.sync.dma_start(out=mskt, in_=msk.rearrange("k p m -> p k m"))

        # PSUM: tag "mm" = 6 rotating banks, tag "av" = 2 banks.
        ps_all = tc.alloc_tile_pool(name="ps_all", bufs=4, space="PSUM")
        wpair = tc.alloc_tile_pool(name="wpair", bufs=3)

        fill_i = [0]

        def emit_fill(n=2, tag="mm"):
            """Dependency-free PE matmuls in ONE psum slot: keep the HAM
            clock gate open through stretches where real PE work is sparse."""
            with nc.named_scope("fill"):
                pw = ps_all.tile([128, 512], f32, tag=tag, bufs=4,
                                 name=f"fw{fill_i[0]}")
                fill_i[0] += 1
                for r in range(n):
                    nc.tensor.matmul(
                        pw, onesf[:, 0:128].bitcast(f32r), onesf.bitcast(f32r),
                        start=(r == 0), stop=(r == n - 1),
                    )

        with nc.named_scope("warmup"):
            for wu in range(11):
                emit_fill(2)

        # x2T = x + attnproj (residual 1), written in the proj phase
        x2t, free_x2t = tc.tile([128, NCI, TQ], bf16, name="x2t")
        # packed normalized heads [128, pair, TQ]
        att2, free_att2 = tc.tile([128, NP, TQ], bf16, name="att2")
        # shared LN scratch (LN1 + LN2) and the attn out-proj weight stream
        ln_sb = tc.alloc_tile_pool(name="ln_sb", bufs=3)
        w1s = tc.alloc_tile_pool(name="w1s", bufs=6)
        wp_pool = tc.alloc_tile_pool(name="wp_pool", bufs=5)

        # ---------------- layernorm building blocks ----------------
        def ln_stats_ci(x_ap, psx, psq, ci, ones_x, tag):
            """One ci block's contribution to the mean / sq-mean chains."""
            nc.tensor.matmul(
                psx, ones_x, x_ap, start=(ci == 0), stop=(ci == NCI - 1)
            )
            sq = ln_sb.tile([128, x_ap.shape[-1]], bf16, tag="sq", name=f"sq{tag}_{ci}")
            if ci % 2 == 0:
                nc.scalar.activation(out=sq, in_=x_ap, func=Act.Square)
            else:
                nc.vector.tensor_mul(out=sq, in0=x_ap, in1=x_ap)
            nc.tensor.matmul(
                psq, ones128b, sq, start=(ci == 0), stop=(ci == NCI - 1)
            )

        def ln_rows(psx, psq, blk_w, tag):
            """rstd (c0) and -mu*rstd (c1) rows from the psx/psq sums."""
            ms = ln_sb.tile([1, blk_w], f32r, tag="rs", bufs=4, name=f"ms{tag}")
            nc.vector.tensor_scalar_mul(ms, psq, 1.0 / C)
            mu2 = ln_sb.tile([1, blk_w], f32r, tag="rs", bufs=4, name=f"mu2{tag}")
            nc.scalar.activation(out=mu2, in_=psx, func=Act.Square, scale=1.0 / C)
            nmu = ln_sb.tile([1, blk_w], f32r, tag="rs", bufs=4, name=f"nmu{tag}")
            nc.vector.tensor_scalar_mul(nmu, psx, -1.0 / C)
            var = ln_sb.tile([1, blk_w], f32r, tag="rs", bufs=4, name=f"var{tag}")
            nc.vector.tensor_sub(out=var, in0=ms, in1=mu2)
            sd = ln_sb.tile([1, blk_w], f32r, tag="rs", bufs=4, name=f"sd{tag}")
            nc.scalar.activation(out=sd, in_=var, func=Act.Ln, bias=epst)
            c0 = ln_sb.tile([1, blk_w], f32r, tag=f"c0_{tag}", bufs=1)
            nc.scalar.activation(out=c0, in_=sd, func=Act.Exp, scale=-0.5)
            c1 = ln_sb.tile([1, blk_w], f32r, tag=f"c1_{tag}", bufs=1)
            nc.vector.tensor_mul(out=c1, in0=nmu, in1=c0)
            return c0, c1

        def ln_apply_ci(x_ap, dst_ap, g_row, bcol, ci, c0, c1, tag,
                        ps_tag="mm", mul_pool=False):
            """dst = x*G0 + beta + G1 with G0 = g(x)c0, G1 = g(x)c1."""
            blk_w = x_ap.shape[-1]
            gsl = slice(128 * ci, 128 * (ci + 1))
            G0 = ps_all.tile([128, blk_w], f32, tag=ps_tag, bufs=4,
                             name=f"G0_{tag}_{ci}")
            nc.tensor.matmul(G0, g_row[:, gsl], c0)
            G1 = ps_all.tile([128, blk_w], f32, tag=ps_tag, bufs=4,
                             name=f"G1_{tag}_{ci}")
            nc.tensor.matmul(G1, g_row[:, gsl], c1)
            tmp = ln_sb.tile([128, blk_w], bf16, tag="tmp", name=f"t{tag}_{ci}")
            nc.vector.tensor_mul(out=tmp, in0=x_ap, in1=G0)
            nc.vector.scalar_tensor_tensor(
                out=dst_ap, in0=tmp, scalar=bcol[:, ci : ci + 1], in1=G1,
                op0=Alu.add, op1=Alu.add,
            )

        # ---------------- attention tiles ----------------
        a1, free_a1 = tc.tile([128, NCI, T], bf16, name="a1", side="right")
        # vt: per key block, per head: 64 v-columns + a ones column (fused
        # softmax denominator row in the AV matmul output).
        vt, free_vt = tc.tile([128, NKB, H, HD + 1], bf16, name="vt", side="right")
        for kb in range(NKB):
            nc.vector.tensor_copy(
                out=vt[:, kb, :, HD : HD + 1], in_=onesf[:, 0:H].unsqueeze(2)
            )
        # att holds, per head, O^T rows 0..HD-1 (unnormalized) and the
        # reciprocal softmax denominator in row 64.
        att, free_att = tc.tile([65, H, TQ], bf16, name="att", side="right")

        wq_r = wq.rearrange("(ci p) c -> p ci c", p=128)
        wk_r = wk.rearrange("(ci p) c -> p ci c", p=128)
        wv_r = wv.rearrange("(ci p) c -> p ci c", p=128)
        w1_r = w1.rearrange("(ci p) f -> p ci f", p=128)
        w2_r = w2.rearrange("(fi p) c -> p fi c", p=128)

        qk_pool = tc.alloc_tile_pool(name="qk_pool", bufs=3, side="right")
        pt_pool = tc.alloc_tile_pool(name="pt_pool", bufs=4, side="right")

        qts, kts, pts, avps, wvts = {}, {}, {}, {}, {}
        wqts, wkts = {}, {}

        def emit_qk_dma(hp):
            """Prefetch Q/K weight slices for head pair hp."""
            if hp >= NP:
                return
            csl = slice(128 * hp, 128 * (hp + 1))
            wqt = wpair.tile([128, NCI, 128], bf16, tag="wq", bufs=2, name=f"wq{hp}")
            nc.sync.dma_start(out=wqt, in_=wq_r[:, :, csl])
            wkt = wpair.tile([128, NCI, 128], bf16, tag="wk", bufs=2, name=f"wk{hp}")
            nc.sync.dma_start(out=wkt, in_=wk_r[:, :, csl])
            wqts[hp], wkts[hp] = wqt, wkt

        def emit_q(hp):
            """Q projection for head pair hp (feature rows 128*hp..)."""
            if hp >= NP:
                return
            with nc.named_scope("qkv"):
                qt = qk_pool.tile([128, TQ], bf16, tag="qt", name=f"qt{hp}")
                pq = ps_all.tile([128, TQ], f32, tag="mm", name=f"pq{hp}")
                for ci in range(NCI):
                    nc.tensor.matmul(
                        pq, wqts[hp][:, ci, :], a1[:, ci, 0:TQ],
                        start=(ci == 0), stop=(ci == NCI - 1),
                    )
                nc.scalar.copy(out=qt, in_=pq)
                qts[hp] = qt

        def emit_k(hp, tb):
            """K projection for head pair hp, token half tb."""
            if hp >= NP:
                return
            with nc.named_scope("qkv"):
                if tb == 0:
                    kts[hp] = qk_pool.tile([128, T], bf16, tag="kt", name=f"kt{hp}")
                sl = slice(512 * tb, 512 * (tb + 1))
                pk = ps_all.tile([128, 512], f32, tag="mm", name=f"pk{hp}_{tb}")
                for ci in range(NCI):
                    nc.tensor.matmul(
                        pk, wkts[hp][:, ci, :], a1[:, ci, sl],
                        start=(ci == 0), stop=(ci == NCI - 1),
                    )
                nc.scalar.copy(out=kts[hp][:, sl], in_=pk)

        def emit_vdma(g):
            """Prefetch the V weight slice for heads 8g..8g+7."""
            if g >= H // 8:
                return
            csl = slice(512 * g, 512 * (g + 1))
            wvt = wpair.tile([128, NCI, 512], bf16, tag="wv", bufs=1, name=f"wv{g}")
            nc.sync.dma_start(out=wvt, in_=wv_r[:, :, csl])
            wvts[g] = wvt

        def emit_vchunk_kb(g, kb):
            """V projection for heads 8g..8g+7, one key block.
            Activations stationary, 512 weight columns moving."""
            if g >= H // 8:
                return
            with nc.named_scope("qkv"):
                kbsl = slice(128 * kb, 128 * (kb + 1))
                pv = ps_all.tile([128, 512], f32, tag="mm", name=f"pv{g}_{kb}")
                for ci in range(NCI):
                    nc.tensor.matmul(
                        pv, a1[:, ci, kbsl], wvts[g][:, ci, :],
                        start=(ci == 0), stop=(ci == NCI - 1),
                    )
                nc.vector.tensor_copy(
                    out=vt[:, kb, 8 * g : 8 * g + 8, 0:HD],
                    in_=pv.rearrange("p (h d) -> p h d", h=8),
                )

        def emit_scores_kb(hp, kb):
            """Scores + exp + causal mask for both heads of pair hp, one
            key block. Mask multiply runs on the Pool engine."""
            if hp >= NP:
                return
            with nc.named_scope("attn"):
                if kb == 0:
                    p0 = pt_pool.tile([128, offs[-1]], bf16, tag="pt", name=f"pt{2 * hp}")
                    p1 = pt_pool.tile([128, offs[-1]], bf16, tag="pt", name=f"pt{2 * hp + 1}")
                    pts[hp] = (p0, p1)
                qt, kt = qts[hp], kts[hp]
                s = cfg.s_kb(kb)
                n = TQ - s
                kbsl = slice(128 * kb, 128 * (kb + 1))
                pss = []
                for idx in range(2):
                    po = idx * HD
                    ps_s = ps_all.tile([128, 512], f32, tag="mm", name=f"sc{hp}_{kb}_{idx}")
                    nc.tensor.matmul(
                        ps_s[:, 0:n],
                        kt[po : po + HD, kbsl],
                        qt[po : po + HD, s:TQ],
                    )
                    pss.append(ps_s)
                for idx in range(2):
                    dst = pts[hp][idx]
                    nc.scalar.activation(
                        out=dst[:, offs[kb] : offs[kb] + n],
                        in_=pss[idx][:, 0:n],
                        func=Act.Exp, scale=scale,
                    )
                    nc.gpsimd.tensor_mul(
                        out=dst[:, offs[kb] : offs[kb] + 128],
                        in0=dst[:, offs[kb] : offs[kb] + 128],
                        in1=mskt[:, kb, :],
                    )

        def emit_av_kb(hp, kb):
            """One key block of the AV accumulation for both heads of pair
            hp (inputs were produced one pair-period earlier)."""
            if hp < 0:
                return
            with nc.named_scope("attn"):
                s = cfg.s_kb(kb)
                for idx in range(2):
                    h = 2 * hp + idx
                    if kb == 0:
                        avps[h] = ps_all.tile(
                            [65, TQ], f32, tag="av", bufs=4, name=f"av{h}"
                        )
                    nc.tensor.matmul(
                        avps[h][:, s:TQ],
                        vt[:, kb, h, :],
                        pts[hp][idx][:, offs[kb] : offs[kb + 1]],
                        start=(kb == 0), stop=(kb == NKB - 1),
                        skip_group_check=True,
                    )

        def emit_av_finish(hp):
            """Copy unnormalized O^T and denominator rows out of PSUM."""
            with nc.named_scope("attn"):
                for idx in range(2):
                    h = 2 * hp + idx
                    nc.scalar.copy(out=att[0:64, h, :], in_=avps[h][0:64, :])
                    nc.vector.tensor_copy(
                        out=att[64:65, h, :], in_=avps[h][64:65, :]
                    )

        def emit_recip(hs, nh):
            """Batch-reciprocal the denominator rows of heads hs..hs+nh-1
            via a DRAM round-trip spreading them over 128 partitions."""
            assert (nh * TQ) % 128 == 0
            with nc.named_scope("attn"):
                hsl = slice(hs, hs + nh)
                fl = nh * TQ // 128
                nc.sync.dma_start(
                    out=dscr_a.rearrange("(o h t) -> o h t", o=1, h=H)[:, hsl, :],
                    in_=att[64:65, hsl, :],
                )
                dwide = pt_pool.tile([128, fl], bf16, tag="dw", bufs=2, name=f"dw{hs}")
                nc.sync.dma_start(
                    out=dwide,
                    in_=dscr_a[hs * TQ : (hs + nh) * TQ].rearrange(
                        "(p f) -> p f", p=128
                    ),
                )
                nc.vector.reciprocal(out=dwide, in_=dwide)
                nc.sync.dma_start(
                    out=dscr_b[hs * TQ : (hs + nh) * TQ].rearrange(
                        "(p f) -> p f", p=128
                    ),
                    in_=dwide,
                )
                nc.sync.dma_start(
                    out=att[64:65, hsl, :],
                    in_=dscr_b.rearrange("(o h t) -> o h t", o=1, h=H)[:, hsl, :],
                )

        def emit_norm(hp):
            """Normalize pair hp's heads by the reciprocal denominators and
            pack them into att2[:, hp, :] (odd head via partition-shifted
            DVE write to partitions 64..127)."""
            with nc.named_scope("attn"):
                for idx in range(2):
                    h = 2 * hp + idx
                    bc = ps_all.tile([64, TQ], f32, tag="mm", name=f"bc{h}")
                    nc.tensor.matmul(bc, oneshi[64:65, :], att[64:65, h, :])
                    psl = slice(64 * idx, 64 * idx + 64)
                    nc.vector.tensor_mul(
                        out=att2[psl, hp, :], in0=att[0:64, h, :], in1=bc
                    )

        # pipeline: scores of pair hp+1, AV of pair hp, V of pair hp+1,
        # and Q/K of pair hp+2 are interleaved at key-block granularity.
        def ln1_post_block(tb):
            if tb == 0:
                for kb in range(NKB // 2):
                    emit_scores_kb(0, kb)
                    emit_vchunk_kb(0, kb)
            else:
                for kb in range(NKB // 2, NKB):
                    emit_scores_kb(0, kb)
                    emit_vchunk_kb(0, kb)
                emit_q(1)
                emit_k(1, 0)
                emit_k(1, 1)

        # ---------------- LN1 over all T tokens ----------------
        with nc.named_scope("ln1"):
            stats1 = []
            for tb in range(NTB):
                sl = slice(tb * cfg.BW, (tb + 1) * cfg.BW)
                stag = "mm" if tb == 0 else "av"
                psx = ps_all.tile([1, cfg.BW], f32, tag=stag, bufs=4, name=f"psx{tb}")
                psq = ps_all.tile([1, cfg.BW], f32, tag=stag, bufs=4, name=f"psq{tb}")
                for ci in range(NCI):
                    ln_stats_ci(
                        raw[:, ci, sl], psx, psq, ci, ones128, f"1{tb}"
                    )
                stats1.append((psx, psq))
                emit_fill(2)
            emit_fill(14, tag="av")
            rows1 = []
            for tb in range(NTB):
                psx, psq = stats1[tb]
                rows1.append(ln_rows(psx, psq, cfg.BW, f"1{tb}"))
            emit_qk_dma(0)
            emit_qk_dma(1)
            emit_vdma(0)
            for tb in range(NTB):
                sl = slice(tb * cfg.BW, (tb + 1) * cfg.BW)
                c0, c1 = rows1[tb]
                # pair-0 Q/K chains accumulate per ci right behind the apply
                with nc.named_scope("qkv"):
                    if tb == 0:
                        qt0 = qk_pool.tile([128, TQ], bf16, tag="qt", name="qt0")
                        kt0 = qk_pool.tile([128, T], bf16, tag="kt", name="kt0")
                        qts[0], kts[0] = qt0, kt0
                        pq0 = ps_all.tile([128, TQ], f32, tag="mm", name="pq0")
                    pk0 = ps_all.tile([128, 512], f32, tag="mm", name=f"pk0_{tb}")
                for ci in range(NCI):
                    ln_apply_ci(
                        raw[:, ci, sl], a1[:, ci, sl], g1r, lb1t,
                        ci, c0, c1, f"1{tb}", ps_tag="av",
                    )
                    with nc.named_scope("qkv"):
                        if tb == 0:
                            nc.tensor.matmul(
                                pq0, wqts[0][:, ci, :], a1[:, ci, 0:TQ],
                                start=(ci == 0), stop=(ci == NCI - 1),
                            )
                        nc.tensor.matmul(
                            pk0, wkts[0][:, ci, :],
                            a1[:, ci, 512 * tb : 512 * (tb + 1)],
                            start=(ci == 0), stop=(ci == NCI - 1),
                        )
                with nc.named_scope("qkv"):
                    if tb == 0:
                        nc.scalar.copy(out=qt0, in_=pq0)
                    nc.scalar.copy(
                        out=kt0[:, 512 * tb : 512 * (tb + 1)], in_=pk0
                    )
                ln1_post_block(tb)

        # ---------------- attention pair loop ----------------
        wpts = {}
        w1ts = {}
        for hp in range(NP):
            for kb in range(NKB):
                emit_scores_kb(hp + 1, kb)
                emit_av_kb(hp, kb)
                if hp == 1 and kb == 0:
                    emit_vdma(1)
                if hp in (2, 3) and kb % 2 == 0:
                    emit_vchunk_kb(1, 4 * (hp - 2) + kb // 2)
                if hp == 7:
                    emit_fill(2)
                    if kb in (1, 3, 5):
                        w1t = w1s.tile(
                            [128, 1024], bf16, tag="w1", bufs=6,
                            name=f"w1g0_{kb // 2}",
                        )
                        nc.sync.dma_start(
                            out=w1t[:, 0:512], in_=w1_r[:, kb // 2, 0:512]
                        )
                        w1ts[(0, kb // 2)] = w1t
                if kb == 1:
                    emit_qk_dma(hp + 2)
                elif kb == 3:
                    emit_q(hp + 2)
                    if 1 <= hp < 7:
                        emit_norm(hp - 1)
                elif kb == 6 and hp == 7:
                    emit_norm(6)
                elif kb == 5:
                    emit_k(hp + 2, 0)
                elif kb == 7:
                    emit_k(hp + 2, 1)
                # prefetch attn out-proj weights during the late pairs
                if hp >= 5 and kb % 2 == 1:
                    ci = (hp - 5) * 4 + kb // 2
                    if ci < NCI:
                        wt = wp_pool.tile([128, C], bf16, tag="w", name=f"wpt{ci}")
                        nc.sync.dma_start(
                            out=wt, in_=wp[128 * ci : 128 * (ci + 1)]
                        )
                        wpts[ci] = wt
            if hp < 6:
                emit_av_finish(hp)
                emit_recip(2 * hp, 2)
            elif hp == 6:
                emit_av_finish(6)
                with nc.named_scope("attn"):
                    for idx in range(2):
                        nc.vector.reciprocal(
                            out=att[64:65, 12 + idx, :],
                            in_=att[64:65, 12 + idx, :],
                        )

        # ---------------- attention tail + out-proj + residual 1 ----------
        # Pair 7: O rows drain via ACT; reciprocal straight off the PSUM
        # denominator rows on DVE (no DRAM round-trip).
        with nc.named_scope("attn"):
            for idx in range(2):
                h = 14 + idx
                nc.scalar.copy(out=att[0:64, h, :], in_=avps[h][0:64, :])
                nc.vector.reciprocal(
                    out=att[64:65, h, :], in_=avps[h][64:65, :]
                )

        # proj part A: co 0..4 x ci 0..6 keeps the PE busy while pair 7's
        # reciprocal + norm resolve (pp[0..4] on "mm"; one mm slot stays
        # free for norm(7)'s bc; pp[5..7] allocated after the "av" banks
        # drain).
        pp = [None] * NCI
        with nc.named_scope("proj"):
            for i in range(4):
                pp[i] = ps_all.tile([128, TQ], f32, tag="mm", bufs=4, name=f"pp{i}")
            for ci in range(7):
                for co in range(4):
                    nc.tensor.matmul(
                        pp[co],
                        wpts[ci][:, 128 * co : 128 * (co + 1)],
                        att2[:, ci, :],
                        start=(ci == 0), stop=False,
                    )
        # norm(7): bc matmuls run in the (already drained) "av" banks so the
        # six open pp accumulators in "mm" are never wrapped onto.
        with nc.named_scope("attn"):
            for idx in range(2):
                h = 14 + idx
                bc = ps_all.tile([64, TQ], f32, tag="av", bufs=4, name=f"bc{h}")
                nc.tensor.matmul(bc, oneshi[64:65, :], att[64:65, h, :])
                psl = slice(64 * idx, 64 * idx + 64)
                nc.vector.tensor_mul(
                    out=att2[psl, 7, :], in0=att[0:64, h, :], in1=bc
                )
        with nc.named_scope("proj"):
            # close pp0..3 first so their epilogues + LN2 stats start early
            for co in range(4):
                nc.tensor.matmul(
                    pp[co],
                    wpts[7][:, 128 * co : 128 * (co + 1)],
                    att2[:, 7, :],
                    start=False, stop=True,
                )
            for i in range(4, 8):
                pp[i] = ps_all.tile([128, TQ], f32, tag="av", bufs=4, name=f"pp{i}")
            for ci in range(7):
                for co in range(4, 8):
                    nc.tensor.matmul(
                        pp[co],
                        wpts[ci][:, 128 * co : 128 * (co + 1)],
                        att2[:, ci, :],
                        start=(ci == 0), stop=False,
                    )
            for co in range(4, 8):
                nc.tensor.matmul(
                    pp[co],
                    wpts[7][:, 128 * co : 128 * (co + 1)],
                    att2[:, 7, :],
                    start=False, stop=True,
                )
        wp_pool.release()

        # proj epilogue interleaved with LN2 stats (per ci as x2t lands)
        psx2 = ps_all.tile([1, TQ], f32, tag="mm", name="psx2")
        psq2 = ps_all.tile([1, TQ], f32, tag="mm", name="psq2")
        def emit_x2t(co):
            nc.vector.scalar_tensor_tensor(
                out=x2t[:, co, :],
                in0=pp[co],
                scalar=bpjt[:, co : co + 1],
                in1=raw[:, co, 0:TQ],
                op0=Alu.add,
                op1=Alu.add,
            )

        with nc.named_scope("proj"):
            emit_x2t(0)
            emit_x2t(1)
            for co in range(NCI):
                if co + 2 < NCI:
                    emit_x2t(co + 2)
                with nc.named_scope("ln2"):
                    ln_stats_ci(
                        x2t[:, co, :], psx2, psq2, co, ones128b, "2"
                    )
                if co >= 3:
                    w1t = w1s.tile(
                        [128, 1024], bf16, tag="w1", bufs=6, name=f"w1g0_{co}"
                    )
                    nc.sync.dma_start(out=w1t[:, 0:512], in_=w1_r[:, co, 0:512])
                    w1ts[(0, co)] = w1t
                if co == 4:
                    emit_fill(22, tag="av")
        pt_pool.release()
        qk_pool.release()
        free_att()
        free_vt()
        free_a1()
        free_raw()

        # ---------------- LN2 rows + apply interleaved with FFN1 group 0 --
        a2, free_a2 = tc.tile([128, NCI, TQ], bf16, name="a2", side="right")
        hsb, free_hsb = tc.tile([128, NFF, TQ], bf16, name="hsb", side="right")
        w2s = tc.alloc_tile_pool(name="w2s", bufs=4)
        with nc.named_scope("ln2"):
            c02, c12 = ln_rows(psx2, psq2, TQ, "2")
            emit_fill(4)
        # FFN1 group 0: ff-blocks 0..5 accumulate per-ci right after each
        # LN2 apply lands that ci; LN G0/G1 run in the "av" banks.
        pf0 = [
            ps_all.tile([128, TQ], f32, tag="mm", name=f"pf0_{i}")
            for i in range(6)
        ]
        with nc.named_scope("ln2"):
            for ci in range(NCI):
                ln_apply_ci(
                    x2t[:, ci, :], a2[:, ci, :], g2r, lb2t,
                    ci, c02, c12, "2", ps_tag="av",
                )
                with nc.named_scope("ffn1"):
                    for co in range(4):
                        nc.tensor.matmul(
                            pf0[co],
                            w1ts[(0, ci)][:, 128 * co : 128 * (co + 1)],
                            a2[:, ci, :],
                            start=(ci == 0), stop=(ci == NCI - 1),
                        )
        x3t, free_x3t = tc.tile([128, NCI, TQ], bf16, name="x3t")
        w2ts = {}
        with nc.named_scope("ffn1"):
            for co in range(4):
                nc.scalar.activation(
                    out=hsb[:, co, :], in_=pf0[co], func=Act.Gelu,
                    bias=b1t[:, co : co + 1],
                )
            # remaining 28 ff-blocks in groups; weights streamed per (g, ci)
            groups = [(4, 8), (12, 8), (20, 8), (28, 4)]
            for gi, (f0, nco) in enumerate(groups):
                pf = [
                    ps_all.tile(
                        [128, TQ], f32,
                        tag=("mm" if i < 4 else "av"), bufs=4,
                        name=f"pf{gi + 1}_{i}",
                    )
                    for i in range(nco)
                ]
                for ci in range(NCI):
                    wt = w1s.tile(
                        [128, 1024], bf16, tag="w1", bufs=6,
                        name=f"w1g{gi + 1}_{ci}",
                    )
                    nc.sync.dma_start(
                        out=wt[:, 0 : 128 * nco],
                        in_=w1_r[:, ci, 128 * f0 : 128 * (f0 + nco)],
                    )
                    for co in range(nco):
                        nc.tensor.matmul(
                            pf[co],
                            wt[:, 128 * co : 128 * (co + 1)],
                            a2[:, ci, :],
                            start=(ci == 0), stop=(ci == NCI - 1),
                        )
                for co in range(nco):
                    hco = f0 + co
                    nc.scalar.activation(
                        out=hsb[:, hco, :], in_=pf[co], func=Act.Gelu,
                        bias=b1t[:, hco : hco + 1],
                    )
                # W2 prefetch + x3t prep spread across the groups
                w2t = w2s.tile([128, 8, C], bf16, tag="w2", bufs=4, name=f"w2t{gi}")
                nc.sync.dma_start(out=w2t, in_=w2_r[:, 8 * gi : 8 * (gi + 1), :])
                w2ts[gi] = w2t
                for k in range(2):
                    ci = 2 * gi + k
                    nc.vector.tensor_scalar_add(
                        x3t[:, ci, :], x2t[:, ci, :], b2t[:, ci : ci + 1]
                    )

        # ---------------- FFN2: resident W2, output-pipelined ----------
        yt_pool = tc.alloc_tile_pool(name="yt_pool", bufs=3, side="right")
        yt_r = yt.rearrange("(ci p) t -> ci p t", p=128)
        with nc.named_scope("ffn2"):
            for co in range(NCI):
                py = ps_all.tile(
                    [128, TQ], f32,
                    tag=("mm" if co % 2 == 0 else "av"), bufs=4,
                    name=f"py{co}",
                )
                csl = slice(128 * co, 128 * (co + 1))
                for fi in range(NFF):
                    nc.tensor.matmul(
                        py,
                        w2ts[fi // 8][:, fi % 8, csl],
                        hsb[:, fi, :],
                        start=(fi == 0), stop=(fi == NFF - 1),
                    )
                yts = yt_pool.tile([128, TQ], f32, tag="y", name=f"yts{co}")
                nc.vector.tensor_add(out=yts, in0=py, in1=x3t[:, co, :])
                nc.sync.dma_start(out=yt_r[co], in_=yts)

        yt_pool.release()
        free_x3t()
        w2s.release()
        free_hsb()
        free_a2()
        w1s.release()
        ln_sb.release()
        free_att2()
        free_x2t()
        wpair.release()
        ps_all.release()
        free_mskt()
        free_b2t()
        free_b1t()
        free_bpjt()
        free_lb2t()
        free_lb1t()
        free_g2r()
        free_g1r()
        free_epst()
        free_oneshi()
        free_ones128b()
        free_ones128()
        free_onesf()

    nc.compile()
    return nc


def prep_core_inputs(cfg: Cfg, inputs: dict, b: int, j: int) -> dict:
    """Host-side slicing/permutation for core (batch b, parity j)."""
    T, TQ, NKB = cfg.T, cfg.TQ, cfg.NKB
    x = np.asarray(inputs["x"])
    perm = np.concatenate([np.arange(j, T, 2), np.arange(1 - j, T, 2)])
    xp = x[b][perm]  # [T, C]
    xpt = np.ascontiguousarray(xp.T, dtype=np.float32)

    import ml_dtypes

    qtok = perm[:TQ]
    ktok = perm
    mask = np.ones((NKB, 128, 128), dtype=np.float32)
    for kb in range(NKB):
        s = cfg.s_kb(kb)
        kt = ktok[128 * kb : 128 * (kb + 1)]  # [128]
        qt = qtok[s : s + 128]  # [128]
        allowed = qt[None, :] >= kt[:, None]  # [128, 128]
        mask[kb] = np.where(allowed, 1.0, 0.0)
    return {"xpt": xpt, "msk": mask.astype(ml_dtypes.bfloat16)}


def prep_shared_inputs(cfg: Cfg, inputs: dict) -> dict:
    import ml_dtypes

    C = cfg.C
    f32 = np.float32
    bf16 = ml_dtypes.bfloat16

    def wq2d(w):  # [H, C, HD] -> [C, H*HD]
        w = np.asarray(w)
        return np.ascontiguousarray(
            w.transpose(1, 0, 2).reshape(C, C)
        ).astype(bf16)

    return {
        "wq": wq2d(inputs["Wq"]),
        "wk": wq2d(inputs["Wk"]),
        "wv": wq2d(inputs["Wv"]),
        "wp": np.ascontiguousarray(inputs["Wproj"]).astype(bf16),
        "w1": np.ascontiguousarray(inputs["W1"]).astype(bf16),
        "w2": np.ascontiguousarray(inputs["W2"]).astype(bf16),
        "ln1g": np.ascontiguousarray(inputs["ln1_g"], dtype=f32),
        "ln1b": np.ascontiguousarray(inputs["ln1_b"], dtype=f32),
        "ln2g": np.ascontiguousarray(inputs["ln2_g"], dtype=f32),
        "ln2b": np.ascontiguousarray(inputs["ln2_b"], dtype=f32),
        "bpj": np.ascontiguousarray(inputs["bproj"], dtype=f32),
        "b1": np.ascontiguousarray(inputs["b1"], dtype=f32),
        "b2": np.ascontiguousarray(inputs["b2"], dtype=f32),
    }


def run(cfg: Cfg, inputs: dict, n_cores: int = 8, trace: bool = False):
    from concourse.bass_utils import run_bass_kernel_spmd

    nc = build_nc(cfg, n_cores=n_cores)
    shared = prep_shared_inputs(cfg, inputs)
    in_maps = []
    cores = []
    for core in range(n_cores):
        b, j = divmod(core, 2)
        b = b % cfg.B
        in_maps.append({**prep_core_inputs(cfg, inputs, b, j), **shared})
        cores.append((b, j))
    res = run_bass_kernel_spmd(
        nc, in_maps, core_ids=list(range(n_cores)), trace=trace
    )
    out = np.zeros((cfg.B, cfg.T, cfg.C), dtype=np.float32)
    for core, (b, j) in enumerate(cores):
        ytv = res.results[core]["yt"]  # [C, TQ]
        perm = np.concatenate(
            [np.arange(j, cfg.T, 2), np.arange(1 - j, cfg.T, 2)]
        )
        out[b, perm[: cfg.TQ], :] = ytv.T
    return out, res


def kernel(**inputs) -> np.ndarray:
    out, _ = run(Cfg(), inputs, n_cores=8, trace=False)
    return out


if __name__ == "__main__":
    rng = np.random.default_rng(0)
    cfg = Cfg()
    ins = {
        "x": rng.standard_normal((cfg.B, cfg.T, cfg.C)).astype(np.float32),
        "ln1_g": np.ones(cfg.C, np.float32),
        "ln1_b": np.zeros(cfg.C, np.float32),
        "ln2_g": np.ones(cfg.C, np.float32),
        "ln2_b": np.zeros(cfg.C, np.float32),
        "Wq": rng.standard_normal((cfg.H, cfg.C, cfg.HD)).astype(np.float32)
        * 0.02,
        "Wk": rng.standard_normal((cfg.H, cfg.C, cfg.HD)).astype(np.float32)
        * 0.02,
        "Wv": rng.standard_normal((cfg.H, cfg.C, cfg.HD)).astype(np.float32)
        * 0.02,
        "Wproj": rng.standard_normal((cfg.C, cfg.C)).astype(np.float32) * 0.02,
        "bproj": np.zeros(cfg.C, np.float32),
        "W1": rng.standard_normal((cfg.C, cfg.FF)).astype(np.float32) * 0.02,
        "b1": np.zeros(cfg.FF, np.float32),
        "W2": rng.standard_normal((cfg.FF, cfg.C)).astype(np.float32) * 0.02,
        "b2": np.zeros(cfg.C, np.float32),
    }
    y = kernel(**ins)
    print("ran, out", y.shape, y.dtype, float(np.abs(y).max()))
